# revision 8
# baseline (speedup 1.0000x reference)
"""Distributed attention kernel for 8 TRN2 NeuronCores (S^T-layout pipeline).

Problem: x[8192,1024] @ {W_q,W_k,W_v}[1024,128] -> softmax(QK^T/sqrt(128)) @ V.

Sharding: x row-sharded (1024 rows/core), weights replicated. Each core
computes K^T_loc/V_loc from its shard, AllGathers K^T (f32r) and V (bf16),
then attends its own 1024 Q rows against the full K/V.

Per-core pipeline (all loops fully unrolled, Tile framework for sync):
  1. x^T via PE transposes (fp32 exact).
  2. Q^T,K^T projections in float32r (11-bit-mantissa fp32; 4x faster PE),
     V projection in bf16. Q^T pre-scaled by 1/sqrt(128).
  3. AllGather K^T + V via DRAM bounce buffers.
  4. Stats phase (per 128-row q-tile): S = Q^T.T @ K^T in bf16 with bf16
     PSUM output; DVE row-max -> m_hat. m_hat only needs to be within ~80
     of the true row max (softmax shift tolerance), bf16 error is ~3.
  5. Main phase (per 512-col q-chunk, per 128-row kv-tile):
       PSUM = K^T_tile.T @ Q^T_chunk (f32r)  [S^T layout: kv on partitions]
       PSUM += ones.T @ (-m_hat row)         [rank-1 max shift]
       A^T = exp(PSUM) via ACT -> bf16 SBUF
       O^T += V_tile.T?? no: O^T[dv,q] += V_tile(lhsT) . A^T   [bf16 matmul]
       den[1,q] += ones128.T . A^T           [denominator matmul]
     Then O^T -> transpose -> scale rows by 1/den -> DMA out.

Numerics: logits have std ~1024 (randn inputs); the softmax is near-one-hot
so the Q/K/S path needs |logit error| << 1. f32r gives ~0.1 abs here; bf16
(err ~3) is only used for the max ESTIMATE, which tolerates +-80.
"""

import os
import sys

import numpy as np

os.environ.setdefault("MYCRO_LOCAL_CACHE", "1")

try:
    import concourse  # noqa: F401
except ImportError:  # pragma: no cover - path fallback for fresh dirs
    for _p in ("/opt/trn_rl_repo", "/root/.axon_site/_ro/trn_rl_repo"):
        if os.path.isdir(_p):
            sys.path.insert(0, _p)
    import concourse  # noqa: F401

import concourse.bass as bass
import concourse.mybir as mybir
import concourse.tile as tile
from concourse import bacc
from concourse.bass_utils import run_bass_kernel_spmd
from concourse.masks import make_identity

F32 = mybir.dt.float32
F32R = mybir.dt.float32r
BF16 = mybir.dt.bfloat16

N_CORES = 8
P = 128
NTOK = 8192
DIN = 1024
DQK = 128
DV = 128
NLOC = NTOK // N_CORES  # 1024 rows per core
TQ = NLOC // P  # 8 q tiles per core
TD = DIN // P  # 8 d_in tiles
NKV = NTOK // P  # 64 kv tiles
QC = 512  # q-chunk width for the main phase
NQC = NLOC // QC  # 2 q-chunks
SCH = 1024  # stats chunk width (bf16 psum bank)
NSCH = NTOK // SCH  # 8 stats chunks per q-tile
SCALE = 1.0 / float(np.sqrt(DQK))

# Matmul PSUM output must be fp32 (bass asserts), so stats maxes read fp32
# PSUM at 1 elem/cycle on DVE.
STATS_BF16_PSUM = False
# AllGather K^T in float32r dtype end-to-end; fallback re-rounds after DMA.
AG_F32R = True


def build_nc():
    nc = bacc.Bacc(
        "TRN2",
        target_bir_lowering=False,
        debug=False,
        enable_asserts=False,
        num_devices=N_CORES,
    )

    x_d = nc.dram_tensor("x", [NLOC, DIN], F32, kind="ExternalInput").ap()
    wq_d = nc.dram_tensor("W_q", [DIN, DQK], F32, kind="ExternalInput").ap()
    wk_d = nc.dram_tensor("W_k", [DIN, DQK], F32, kind="ExternalInput").ap()
    wv_d = nc.dram_tensor("W_v", [DIN, DV], F32, kind="ExternalInput").ap()
    out_d = nc.dram_tensor("out", [NLOC, DV], F32, kind="ExternalOutput").ap()

    groups = [list(range(N_CORES))]
    kt_dt = F32R if AG_F32R else F32

    with tile.TileContext(nc) as tc:
        with (
            tc.tile_pool(name="consts", bufs=1) as consts,
            tc.tile_pool(name="persist", bufs=1) as persist,
            tc.tile_pool(name="dram", bufs=1, space="DRAM") as dram,
        ):
            ident_f32 = consts.tile([P, P], F32)
            make_identity(nc, ident_f32)
            ones_f = consts.tile([1, P], F32)
            nc.vector.memset(ones_f, 1.0)
            ones_r = consts.tile([1, P], F32R)
            nc.vector.tensor_copy(out=ones_r, in_=ones_f)
            ones_col_bf = consts.tile([P, 1], BF16)
            nc.vector.memset(ones_col_bf, 1.0)

            # Persistent SBUF tensors.
            qT = persist.tile([P, NLOC], F32R)  # Q^T, pre-scaled, f32r
            qT_bf = persist.tile([P, NLOC], BF16)
            kT_full = persist.tile([P, NTOK], kt_dt)
            kT_bf = persist.tile([P, NTOK], BF16)
            vf = persist.tile([P, NKV, P], BF16)  # gathered V tiles
            kTl = persist.tile([P, NLOC], kt_dt)
            vl = persist.tile([P, TQ, P], BF16)
            negm_row = persist.tile([1, NLOC], F32R)

            # DRAM bounce buffers for the collectives.
            kT_bounce = dram.tile([P, NLOC], kt_dt)
            kT_gath = dram.tile([P * N_CORES, NLOC], kt_dt, addr_space="Shared")
            v_bounce = dram.tile([NLOC, DV], BF16)
            v_gath = dram.tile([NTOK, DV], BF16, addr_space="Shared")

            # ---------------- projections ----------------
            with (
                tc.tile_pool(name="proj_sb", bufs=1) as proj_sb,
                tc.tile_pool(name="ps_xt", bufs=2, space="PSUM") as ps_xt_pool,
                tc.tile_pool(name="ps_mm", bufs=2, space="PSUM") as ps_mm_pool,
                tc.tile_pool(name="ps_v", bufs=2, space="PSUM") as ps_v_pool,
            ):
                xa = proj_sb.tile([P, TQ, DIN], F32)
                xT_r = proj_sb.tile([P, TD, NLOC], F32R)
                xT_bf = proj_sb.tile([P, TD, NLOC], BF16)
                wq = proj_sb.tile([P, TD, DQK], F32)
                wk = proj_sb.tile([P, TD, DQK], F32)
                wv = proj_sb.tile([P, TD, DV], F32)
                wq_r = proj_sb.tile([P, TD, DQK], F32R)
                wk_r = proj_sb.tile([P, TD, DQK], F32R)
                wv_bf = proj_sb.tile([P, TD, DV], BF16)

                with nc.named_scope("load"):
                    for tj in range(TQ):
                        nc.sync.dma_start(
                            out=xa[:, tj, :], in_=x_d[tj * P : (tj + 1) * P, :]
                        )
                    nc.sync.dma_start(
                        out=wq, in_=wq_d.rearrange("(t p) d -> p t d", p=P)
                    )
                    nc.sync.dma_start(
                        out=wk, in_=wk_d.rearrange("(t p) d -> p t d", p=P)
                    )
                    nc.sync.dma_start(
                        out=wv, in_=wv_d.rearrange("(t p) d -> p t d", p=P)
                    )
                    nc.vector.tensor_copy(out=wq_r, in_=wq)
                    nc.vector.tensor_copy(out=wk_r, in_=wk)
                    nc.vector.tensor_copy(out=wv_bf, in_=wv)

                with nc.named_scope("xT"):
                    # x^T via PE transposes, batching 4 into a PSUM bank.
                    for di in range(TD):
                        for tg in range(TQ // 4):
                            ps_xt = ps_xt_pool.tile([P, 4 * P], F32, tag="ps_xt")
                            for j in range(4):
                                tj = tg * 4 + j
                                nc.tensor.transpose(
                                    ps_xt[:, j * P : (j + 1) * P],
                                    xa[:, tj, di * P : (di + 1) * P],
                                    ident_f32,
                                )
                            nc.vector.tensor_copy(
                                out=xT_r[:, di, tg * 4 * P : (tg + 1) * 4 * P],
                                in_=ps_xt,
                            )
                            nc.vector.tensor_copy(
                                out=xT_bf[:, di, tg * 4 * P : (tg + 1) * 4 * P],
                                in_=ps_xt,
                            )

                with nc.named_scope("kT_proj"):
                    for h in range(NLOC // 512):
                        ps_k = ps_mm_pool.tile([P, 512], F32, tag="ps_mm")
                        for di in range(TD):
                            nc.tensor.matmul(
                                ps_k,
                                wk_r[:, di, :],
                                xT_r[:, di, h * 512 : (h + 1) * 512],
                                start=(di == 0),
                                stop=(di == TD - 1),
                            )
                        nc.vector.tensor_copy(
                            out=kTl[:, h * 512 : (h + 1) * 512], in_=ps_k
                        )
                    nc.sync.dma_start(out=kT_bounce, in_=kTl)

                with nc.named_scope("v_proj"):
                    for tj in range(TQ):
                        ps_v = ps_v_pool.tile([P, DV], F32, tag="ps_v")
                        for di in range(TD):
                            nc.tensor.matmul(
                                ps_v,
                                xT_bf[:, di, tj * P : (tj + 1) * P],
                                wv_bf[:, di, :],
                                start=(di == 0),
                                stop=(di == TD - 1),
                            )
                        nc.vector.tensor_copy(out=vl[:, tj, :], in_=ps_v)
                        nc.sync.dma_start(
                            out=v_bounce[tj * P : (tj + 1) * P, :], in_=vl[:, tj, :]
                        )

                with nc.named_scope("allgather"):
                    nc.gpsimd.collective_compute(
                        "AllGather",
                        mybir.AluOpType.bypass,
                        replica_groups=groups,
                        ins=[kT_bounce.opt()],
                        outs=[kT_gath.opt()],
                    )
                    nc.gpsimd.collective_compute(
                        "AllGather",
                        mybir.AluOpType.bypass,
                        replica_groups=groups,
                        ins=[v_bounce.opt()],
                        outs=[v_gath.opt()],
                    )

                with nc.named_scope("q_proj"):
                    for h in range(NLOC // 512):
                        ps_q = ps_mm_pool.tile([P, 512], F32, tag="ps_mm")
                        for di in range(TD):
                            nc.tensor.matmul(
                                ps_q,
                                wq_r[:, di, :],
                                xT_r[:, di, h * 512 : (h + 1) * 512],
                                start=(di == 0),
                                stop=(di == TD - 1),
                            )
                        nc.vector.tensor_scalar_mul(
                            qT[:, h * 512 : (h + 1) * 512], ps_q, SCALE
                        )
                    nc.vector.tensor_copy(out=qT_bf, in_=qT)

                with nc.named_scope("gather_in"):
                    for c in range(N_CORES):
                        nc.sync.dma_start(
                            out=kT_full[:, c * NLOC : (c + 1) * NLOC],
                            in_=kT_gath[c * P : (c + 1) * P, :],
                        )
                        nc.sync.dma_start(
                            out=vf[:, c * TQ : (c + 1) * TQ, :],
                            in_=v_gath[c * NLOC : (c + 1) * NLOC, :].rearrange(
                                "(t p) d -> p t d", p=P
                            ),
                        )
                    if not AG_F32R:
                        # re-round the gathered fp32 K^T to f32r in place
                        nc.vector.tensor_copy(
                            out=kT_full.bitcast(F32R), in_=kT_full
                        )
                    nc.vector.tensor_copy(out=kT_bf, in_=kT_full.bitcast(F32))

            # ---------------- attention ----------------
            with (
                tc.tile_pool(name="attn_sb", bufs=3) as attn_sb,
                tc.tile_pool(name="stat_sb", bufs=2) as stat_sb,
                tc.tile_pool(name="ps_stat", bufs=2, space="PSUM") as ps_stat_pool,
                tc.tile_pool(name="ps_st", bufs=3, space="PSUM") as ps_st_pool,
                tc.tile_pool(name="ps_o", bufs=1, space="PSUM") as ps_o_pool,
                tc.tile_pool(name="ps_den", bufs=1, space="PSUM") as ps_den_pool,
            ):
                # ---- stats: per q-tile row-max estimate in bf16 ----
                stat_dt = BF16 if STATS_BF16_PSUM else F32
                stat_w = SCH if STATS_BF16_PSUM else 512
                n_stat = NTOK // stat_w
                for qt in range(TQ):
                    with nc.named_scope(f"stats_{qt}"):
                        mx = stat_sb.tile([P, n_stat], F32, tag="mx")
                        for ch in range(n_stat):
                            ps_stat = ps_stat_pool.tile(
                                [P, stat_w], stat_dt, tag="ps_stat"
                            )
                            nc.tensor.matmul(
                                ps_stat,
                                qT_bf[:, qt * P : (qt + 1) * P],
                                kT_bf[:, ch * stat_w : (ch + 1) * stat_w],
                                start=True,
                                stop=True,
                            )
                            nc.vector.reduce_max(
                                mx[:, ch : ch + 1],
                                ps_stat,
                                axis=mybir.AxisListType.X,
                            )
                        m1 = stat_sb.tile([P, 1], F32, tag="m1")
                        negm = stat_sb.tile([P, 1], F32, tag="negm")
                        nc.vector.reduce_max(m1, mx, axis=mybir.AxisListType.X)
                        nc.vector.tensor_scalar_mul(negm, m1, -1.0)
                        # transpose negm [128,1] -> [1,128] into negm_row
                        ps_nm = ps_den_pool.tile([1, P], F32, tag="ps_nm")
                        nc.tensor.transpose(ps_nm, negm, ident_f32)
                        nc.vector.tensor_copy(
                            out=negm_row[0:1, qt * P : (qt + 1) * P], in_=ps_nm
                        )

                # ---- main phase: per q-chunk of 512 ----
                for qc in range(NQC):
                    qs = qc * QC
                    with nc.named_scope(f"main_{qc}"):
                        ps_o = ps_o_pool.tile([P, QC], F32, tag="ps_o")
                        ps_den = ps_den_pool.tile([1, QC], F32, tag="ps_den")
                        for kv in range(NKV):
                            ps_st = ps_st_pool.tile([P, QC], F32, tag="ps_st")
                            nc.tensor.matmul(
                                ps_st,
                                kT_full[:, kv * P : (kv + 1) * P],
                                qT[:, qs : qs + QC],
                                start=True,
                                stop=False,
                            )
                            nc.tensor.matmul(
                                ps_st,
                                ones_r,
                                negm_row[0:1, qs : qs + QC],
                                start=False,
                                stop=True,
                            )
                            aT = attn_sb.tile([P, QC], BF16, tag="aT")
                            nc.scalar.activation(
                                aT,
                                ps_st,
                                mybir.ActivationFunctionType.Exp,
                            )
                            nc.tensor.matmul(
                                ps_o,
                                vf[:, kv, :],
                                aT,
                                start=(kv == 0),
                                stop=(kv == NKV - 1),
                            )
                            nc.tensor.matmul(
                                ps_den,
                                ones_col_bf,
                                aT,
                                start=(kv == 0),
                                stop=(kv == NKV - 1),
                            )

                    with nc.named_scope(f"out_{qc}"):
                        # reciprocal of denominator, transposed to [128, 4]
                        den_row = stat_sb.tile([1, QC], F32, tag="den_row")
                        rden_row = stat_sb.tile([1, QC], F32, tag="rden_row")
                        nc.vector.tensor_copy(out=den_row, in_=ps_den)
                        nc.vector.reciprocal(rden_row, den_row)
                        ps_rd = ps_den_pool.tile([P, QC // P], F32, tag="ps_nm")
                        for j in range(QC // P):
                            nc.tensor.transpose(
                                ps_rd[:, j : j + 1],
                                rden_row[0:1, j * P : (j + 1) * P],
                                ones_f[0:1, 0:1],
                            )
                        rden_col = stat_sb.tile([P, QC // P], F32, tag="rden_col")
                        nc.vector.tensor_copy(out=rden_col, in_=ps_rd)

                        # O^T [dv, q] -> sbuf -> transpose to [q, dv] -> scale
                        oT_sb = stat_sb.tile([P, QC], F32, tag="oT_sb")
                        nc.vector.tensor_copy(out=oT_sb, in_=ps_o)
                        o_nat = stat_sb.tile([P, QC // P, DV], F32, tag="o_nat")
                        ps_on = ps_st_pool.tile([P, QC], F32, tag="ps_st")
                        for j in range(QC // P):
                            nc.tensor.transpose(
                                ps_on[:, j * P : (j + 1) * P],
                                oT_sb[:, j * P : (j + 1) * P],
                                ident_f32,
                            )
                        for j in range(QC // P):
                            nc.vector.tensor_scalar_mul(
                                o_nat[:, j, :],
                                ps_on[:, j * P : (j + 1) * P],
                                rden_col[:, j : j + 1],
                            )
                        nc.sync.dma_start(
                            out=out_d[qs : qs + QC, :].rearrange(
                                "(t p) d -> p t d", p=P
                            ),
                            in_=o_nat,
                        )

    nc.compile()
    return nc


_NC_CACHE = None


def _get_nc():
    global _NC_CACHE
    if _NC_CACHE is None:
        _NC_CACHE = build_nc()
    return _NC_CACHE


def run(inputs, trace=False, **kw):
    """Run the SPMD kernel; returns BassKernelResults."""
    nc = _get_nc()
    x = np.asarray(inputs["x"], dtype=np.float32)
    wq = np.asarray(inputs["W_q"], dtype=np.float32)
    wk = np.asarray(inputs["W_k"], dtype=np.float32)
    wv = np.asarray(inputs["W_v"], dtype=np.float32)
    in_maps = [
        {
            "x": np.ascontiguousarray(x[c * NLOC : (c + 1) * NLOC]),
            "W_q": wq,
            "W_k": wk,
            "W_v": wv,
        }
        for c in range(N_CORES)
    ]
    return run_bass_kernel_spmd(
        nc, in_maps, core_ids=list(range(N_CORES)), trace=trace, **kw
    )


def kernel(**inputs):
    res = run(inputs, trace=False)
    return np.concatenate([res.results[c]["out"] for c in range(N_CORES)], axis=0)
